# revision 20
# baseline (speedup 1.0000x reference)
"""Causal self-attention (B=2, T=2048, C=1024, H=16) on 8 trn2 NeuronCores.

Sharding: 16 heads / 8 cores = 2 heads per core (both batches on every core).
Per core, for its head pair (h0 at partitions 0-63, h1 at 64-127):
  - QKV projection of the full sequence (384 weight columns), producing
    qT/kT in [head_dim, T] layout and V' in [T, head_dim] layout via PE
    transposes, with a ones column appended per head (softmax denominator).
  - Flash-style causal attention on-chip: the two heads' S^T tiles are
    computed as concurrent row-group matmuls into one 2-bank PSUM tile,
    one ACT exp covers both heads, causal masking multiplies a triangular
    0/1 mask on diagonal tiles only (GpSimd), att@V accumulates per head
    with the ones column yielding the denominator row.
  - Softmax normalization: per-head denominator rows are inverted on DVE
    (f32r) and broadcast across partitions with a K=1 PE matmul against a
    ones column; ytb = po * bcast on DVE.  No DRAM round-trip.
  - Output projection as matmul pairs one block behind attention.
Host glue: transpose/cast x to fp16, slice weights per core, sum the 8
fp16 partial outputs in f64, add b_proj.

All matmul operands are fp16 (1 cycle/row on the PE at any tile size, vs
~1.9 for f32r and 4x for f32); PSUM accumulation stays fp32.  Attention
blocks run largest-first, with batch-1 QKV tiles interleaved inside the
first (longest) attention block so the PE never idles and the kernel
tail is the smallest block.
"""

import sys

sys.path.insert(0, "/opt/trn_rl_repo")

import numpy as np

B, T, C, H, HD = 2, 2048, 1024, 16, 64
BT = B * T
NCORE = 8
HPC = H // NCORE  # heads per core
NT = BT // 512    # T-tiles for qkv projection
CCH = C // 128    # contraction chunks

_CACHE = {}


def _build():
    if "nc" in _CACHE:
        return _CACHE["nc"]
    from contextlib import ExitStack

    import concourse.bass as bass
    import concourse.bacc as bacc
    import concourse.mybir as mybir
    import concourse.tile as tile
    from concourse.masks import make_identity, make_upper_triangular

    f32, f32r, f16 = mybir.dt.float32, mybir.dt.float32r, mybir.dt.float16
    AF = mybir.ActivationFunctionType

    nc = bacc.Bacc(None, target_bir_lowering=False, debug=False)
    # x pre-permuted on host to the exact SBUF image [p, tt, cc, t] so each
    # T-tile DMA reads contiguous runs per partition
    xT_d = nc.dram_tensor("xT", [128, NT, CCH, 512], f16, kind="ExternalInput")
    wqkv_d = nc.dram_tensor("wqkv", [128, CCH, 3 * 128], f16, kind="ExternalInput")
    bqkv_d = nc.dram_tensor("bqkv", [128, 3], f32, kind="ExternalInput")
    wp_d = nc.dram_tensor("wp", [128, C], f16, kind="ExternalInput")
    out_d = nc.dram_tensor("out", [BT, C], f16, kind="ExternalOutput")

    with tile.TileContext(nc) as tc, ExitStack() as ctx:
        sb = ctx.enter_context(tc.tile_pool(name="sb", bufs=1))
        xp = ctx.enter_context(tc.tile_pool(name="xp", bufs=3))
        vtp = ctx.enter_context(tc.tile_pool(name="vtp", bufs=2))
        esp = ctx.enter_context(tc.tile_pool(name="esp", bufs=6))
        ytp = ctx.enter_context(tc.tile_pool(name="ytp", bufs=3))
        dnp = ctx.enter_context(tc.tile_pool(name="dnp", bufs=4))
        outp = ctx.enter_context(tc.tile_pool(name="outp", bufs=3))
        # PSUM: pa 'mm' 2x[128,512]f32 = 2 banks, pss 2x[128,1024] = 4,
        # pso 2x[65,512] = 2 -> 8 banks total.  All transient tiles (qkv
        # chains, V transposes, proj, denom broadcast) share the 'mm' ring
        # so interleaved qkv tiles never wait on live po accumulators.
        pa = ctx.enter_context(tc.tile_pool(name="pa", bufs=2, space="PSUM"))
        pss = ctx.enter_context(tc.tile_pool(name="pss", bufs=2, space="PSUM"))
        pso = ctx.enter_context(tc.tile_pool(name="pso", bufs=2, space="PSUM"))

        wq_sb = sb.tile([128, CCH, 3 * 128], f16, tag="wq")

        qT = [sb.tile([128, T], f16, tag=f"qT{b}", name=f"qT{b}") for b in range(B)]
        kT = [sb.tile([128, T], f16, tag=f"kT{b}", name=f"kT{b}") for b in range(B)]
        # V' per batch: per k-tile [V_h0 (64) | 1 | V_h1 (64) | 1] = 130 cols;
        # the ones column accumulates the softmax denominator during att@v
        VW = 130
        Vp = [
            sb.tile([128, 16, VW], f16, tag=f"Vp{b}", name=f"Vp{b}") for b in range(B)
        ]
        # ones columns (64, 129) and finite pad from blanket 1.0 fill; V
        # columns are overwritten by the transpose copies.  Vp[1]'s memset is
        # deferred into the first batch-1 qkv tile to keep DVE free at start.
        nc.vector.memset(Vp[0][:, :, :], 1.0)

        # ---------------- QKV projection ----------------
        def emit_qkv_tile(tt):
            b = tt // (NT // B)
            tloc = (tt % (NT // B)) * 512
            if tt == NT // B:
                nc.vector.memset(Vp[1][:, :, :], 1.0)
            x_t = xp.tile([128, CCH, 512], f16, tag="x", name=f"x{tt}")
            if tt == 0:
                # chunked loads on two queue engines so weight and x chunks
                # stream in parallel and the first matmul starts early
                for cc in range(0, CCH, 2):
                    nc.sync.dma_start(
                        out=wq_sb[:, cc : cc + 2, :], in_=wqkv_d[:, cc : cc + 2, :]
                    )
                    nc.gpsimd.dma_start(
                        out=x_t[:, cc : cc + 2, :], in_=xT_d[:, 0, cc : cc + 2, :]
                    )
            else:
                nc.gpsimd.dma_start(out=x_t, in_=xT_d[:, tt, :, :])
            if tt == 0:
                # constants not needed until later: emit their loads after
                # the first x tile so the first matmul starts sooner
                nc.sync.dma_start(out=bias_sb, in_=bqkv_d[:, :])
                nc.sync.dma_start(out=wp_sb, in_=wp_d[:, :])
                make_identity(nc, ident)
                make_upper_triangular(nc, tri2[:, 0, :], val=1.0, diag=True)
                nc.gpsimd.tensor_copy(tri2[:, 1, :], tri2[:, 0, :])
                nc.vector.memset(ones1[:, :], 1.0)
            for g in range(3):
                ps = pa.tile([128, 512], f32, tag="mm", name="ps")
                for cc in range(CCH):
                    nc.tensor.matmul(
                        ps,
                        wq_sb[:, cc, g * 128 : (g + 1) * 128],
                        x_t[:, cc, :],
                        start=(cc == 0),
                        stop=(cc == CCH - 1),
                    )
                if g == 0:
                    nc.vector.tensor_scalar_add(
                        qT[b][:, tloc : tloc + 512], ps, bias_sb[:, 0:1]
                    )
                elif g == 1:
                    nc.vector.tensor_scalar_add(
                        kT[b][:, tloc : tloc + 512], ps, bias_sb[:, 1:2]
                    )
                else:
                    v_t = vtp.tile([128, 512], f16, tag="v")
                    nc.vector.tensor_scalar_add(v_t, ps, bias_sb[:, 2:3])
                    for j in range(4):
                        pt = pa.tile([128, 128], f16, tag="mm", name="pt")
                        nc.tensor.transpose(pt, v_t[:, j * 128 : (j + 1) * 128], ident)
                        ktl = (tt % (NT // B)) * 4 + j
                        # one strided copy moves both heads' V columns
                        nc.vector.tensor_copy(
                            Vp[b][:, ktl, 0:130].rearrange("p (s e) -> p s e", s=2)[
                                :, :, 0:64
                            ],
                            pt[:, :].rearrange("p (s e) -> p s e", s=2),
                        )

        bias_sb = sb.tile([128, 3], f32, tag="bias")
        wp_sb = sb.tile([128, C], f16, tag="wp")
        ident = sb.tile([128, 128], f16, tag="ident")
        tri2 = sb.tile([128, 2, 128], f16, tag="tri2")
        ones1 = sb.tile([1, 64], f16, tag="ones1")

        # ------------- attention + output projection -------------
        scale = 1.0 / 8.0  # 1/sqrt(HD)
        pending_proj = []

        def emit_proj(ytb, b, qb):
            for j in range(4):
                out_t = outp.tile([128, C], f16, tag="out", name="out_t")
                js = slice(j * 128, (j + 1) * 128)
                for ncol in range(2):
                    cs = slice(ncol * 512, (ncol + 1) * 512)
                    pp = pa.tile([128, 512], f32, tag="mm", name="pp")
                    nc.tensor.matmul(
                        pp, ytb[:, js], wp_sb[:, cs], start=True, stop=True
                    )
                    # GpSimd cannot read PSUM; evacuate each PSUM tile as two
                    # half-copies on DVE and ACT in parallel so the 'mm' slot
                    # frees at half the latency and the PE ring never stalls
                    h0 = slice(ncol * 512, ncol * 512 + 256)
                    h1 = slice(ncol * 512 + 256, (ncol + 1) * 512)
                    nc.vector.tensor_copy(out_t[:, h0], pp[:, 0:256])
                    nc.scalar.copy(out_t[:, h1], pp[:, 256:512])
                row = b * T + qb * 512 + j * 128
                nc.sync.dma_start(out=out_d[row : row + 128, :], in_=out_t)

        def emit_att_block(b, qb, fillers=()):
            fillers = list(fillers)
            n_kt = 4 * (qb + 1)
            po = [
                pso.tile([65, 512], f32, tag="po", name=f"po{b}{qb}{h}")
                for h in range(2)
            ]
            pend = []  # att@v pipelined two k-tiles behind S/exp
            for lkt in range(n_kt):
                r0 = max(0, (lkt - 4 * qb) * 128)
                ks = slice(lkt * 128, (lkt + 1) * 128)
                qs = slice(qb * 512 + r0, (qb + 1) * 512)
                ps2 = pss.tile([128, 1024], f32, tag="s2")
                nc.tensor.matmul(
                    ps2[:, r0:512], kT[b][0:64, ks], qT[b][0:64, qs],
                    start=True, stop=True,
                )
                nc.tensor.matmul(
                    ps2[:, 512 + r0 : 1024], kT[b][64:128, ks], qT[b][64:128, qs],
                    start=True, stop=True,
                )
                es = esp.tile([128, 1024], f16, tag="es")
                if r0:
                    nc.scalar.activation(
                        es[:, :].rearrange("p (h q) -> p h q", h=2)[:, :, r0:512],
                        ps2[:, :].rearrange("p (h q) -> p h q", h=2)[:, :, r0:512],
                        AF.Exp,
                        scale=scale,
                    )
                else:
                    nc.scalar.activation(es, ps2, AF.Exp, scale=scale)
                if lkt >= 4 * qb:  # diagonal tile: causal mask, both heads
                    nc.gpsimd.tensor_mul(
                        es[:, :].rearrange("p (h q) -> p h q", h=2)[
                            :, :, r0 : r0 + 128
                        ],
                        es[:, :].rearrange("p (h q) -> p h q", h=2)[
                            :, :, r0 : r0 + 128
                        ],
                        tri2[:, :, :],
                    )
                if len(pend) >= 3:
                    for mm in pend.pop(0):
                        nc.tensor.matmul(**mm)
                    if fillers and lkt % 3 == 2:
                        emit_qkv_tile(fillers.pop(0))
                pend.append(
                    [
                        dict(
                            out=po[h][:, r0:512],
                            lhsT=Vp[b][:, lkt, h * 65 : (h + 1) * 65],
                            rhs=es[:, h * 512 + r0 : (h + 1) * 512],
                            start=(lkt == 0),
                            stop=(lkt == n_kt - 1),
                        )
                        for h in range(2)
                    ]
                )
            for grp in pend:
                for mm in grp:
                    nc.tensor.matmul(**mm)
            for f in fillers:
                emit_qkv_tile(f)

            # previous block's projection runs on the PE while DVE inverts
            # this block's denominators
            if pending_proj:
                emit_proj(*pending_proj.pop())

            # softmax normalization: invert the denominator rows, broadcast
            # across partitions with a K=1 matmul, scale y on the copy out
            # custom-DVE reciprocal needs SBUF fp32 input (and partition
            # offsets must be 32-aligned, so two separate tiles)
            den0 = dnp.tile([1, 512], f32, tag="den0", name="den0")
            den1 = dnp.tile([1, 512], f32, tag="den1", name="den1")
            nc.vector.tensor_copy(den0, po[0][64:65, :])
            nc.vector.tensor_copy(den1, po[1][64:65, :])
            rcf0 = dnp.tile([1, 512], f32, tag="rcf0", name="rcf0")
            rcf1 = dnp.tile([1, 512], f32, tag="rcf1", name="rcf1")
            nc.vector.reciprocal_approx_fast(out=rcf0, in_=den0)
            nc.vector.reciprocal_approx_fast(out=rcf1, in_=den1)
            rc0 = dnp.tile([1, 512], f16, tag="rc", name="rc0")
            rc1 = dnp.tile([1, 512], f16, tag="rc", name="rc1")
            nc.vector.tensor_copy(rc0, rcf0)
            nc.vector.tensor_copy(rc1, rcf1)
            bc = pa.tile([128, 512], f32, tag="mm", name="bc")
            nc.tensor.matmul(bc[0:64, :], ones1[0:1, :], rc0, start=True, stop=True)
            nc.tensor.matmul(bc[64:128, :], ones1[0:1, :], rc1, start=True, stop=True)
            # DVE cannot read two PSUM operands; stage the broadcast in SBUF
            bcs = dnp.tile([128, 512], f16, tag="bcs", name="bcs")
            nc.vector.tensor_copy(bcs, bc)
            ytb = ytp.tile([128, 512], f16, tag="ytb", name="ytb")
            nc.vector.tensor_mul(ytb[0:64, :], po[0][0:64, :], bcs[0:64, :])
            nc.vector.tensor_mul(ytb[64:128, :], po[1][0:64, :], bcs[64:128, :])
            pending_proj.append((ytb, b, qb))

        # Schedule: batch-0 qkv, then attention blocks largest-first with
        # batch-1 qkv tiles interleaved inside the first (16 k-tile) block.
        for tt in range(4):
            emit_qkv_tile(tt)
        emit_att_block(0, 3, fillers=(4, 5, 6, 7))
        emit_att_block(1, 3)
        emit_att_block(0, 2)
        emit_att_block(1, 2)
        emit_att_block(0, 1)
        emit_att_block(1, 1)
        emit_att_block(0, 0)
        emit_att_block(1, 0)
        emit_proj(*pending_proj.pop())

    nc.finalize()
    _CACHE["nc"] = nc
    return nc


def _prep_inputs(x, w_attn, b_attn, w_proj):
    x = np.ascontiguousarray(np.asarray(x, dtype=np.float32))
    w_attn = np.asarray(w_attn, dtype=np.float32)
    b_attn = np.asarray(b_attn, dtype=np.float32)
    w_proj = np.asarray(w_proj, dtype=np.float32)

    # [p, tt, cc, t] image: xT[p, tt, cc, t] = x_flat[tt*512+t, cc*128+p]
    xT = np.ascontiguousarray(
        x.reshape(NT, 512, CCH, 128).transpose(3, 0, 2, 1).astype(np.float16)
    )
    in_maps = []
    for c in range(NCORE):
        hs = [HPC * c + j for j in range(HPC)]
        blocks = []
        bias_cols = []
        for off in (0, C, 2 * C):
            for h in hs:
                blocks.append(w_attn[:, off + h * HD : off + (h + 1) * HD])
            bias_cols.append(
                np.concatenate([b_attn[off + h * HD : off + (h + 1) * HD] for h in hs])
            )
        wq_flat = np.concatenate(blocks, axis=1).astype(np.float16)  # [C, 384]
        wqkv = np.ascontiguousarray(wq_flat.reshape(CCH, 128, 3 * 128).transpose(1, 0, 2))
        bqkv = np.ascontiguousarray(np.stack(bias_cols, axis=1), dtype=np.float32)
        wp = np.ascontiguousarray(
            np.concatenate([w_proj[h * HD : (h + 1) * HD, :] for h in hs], axis=0)
        ).astype(np.float16)  # [128, C]
        in_maps.append({"xT": xT, "wqkv": wqkv, "bqkv": bqkv, "wp": wp})
    return in_maps


def _run(x, w_attn, b_attn, w_proj, b_proj, trace=False, tmpdir=None):
    from concourse.bass_utils import run_bass_kernel_spmd

    nc = _build()
    in_maps = _prep_inputs(x, w_attn, b_attn, w_proj)
    res = run_bass_kernel_spmd(
        nc, in_maps, list(range(NCORE)), trace=trace, tmpdir=tmpdir
    )
    acc = np.sum(
        np.stack([res.results[i]["out"] for i in range(NCORE)]), axis=0, dtype=np.float64
    )
    out = (acc + np.asarray(b_proj, dtype=np.float64)).astype(np.float32)
    return out.reshape(B, T, C), res


def kernel(x, w_attn, b_attn, w_proj, b_proj):
    out, _ = _run(x, w_attn, b_attn, w_proj, b_proj, trace=False)
    return out


# revision 22
# speedup vs baseline: 1.0745x; 1.0745x over previous
"""Causal self-attention (B=2, T=2048, C=1024, H=16) on 8 trn2 NeuronCores.

Sharding: 16 heads / 8 cores = 2 heads per core (both batches on every core).
Per core, for its head pair (h0 at partitions 0-63, h1 at 64-127):
  - QKV projection of the full sequence (384 weight columns), producing
    qT/kT in [head_dim, T] layout and V' in [T, head_dim] layout via PE
    transposes, with a ones column appended per head (softmax denominator).
  - Flash-style causal attention on-chip: the two heads' S^T tiles are
    computed as concurrent row-group matmuls into one 2-bank PSUM tile,
    one ACT exp covers both heads, causal masking multiplies a triangular
    0/1 mask on diagonal tiles only (GpSimd), att@V accumulates per head
    with the ones column yielding the denominator row.
  - Softmax normalization: per-head denominator rows are inverted on DVE
    (f32r) and broadcast across partitions with a K=1 PE matmul against a
    ones column; ytb = po * bcast on DVE.  No DRAM round-trip.
  - Output projection as matmul pairs one block behind attention.
Host glue: transpose/cast x to fp16, slice weights per core, sum the 8
fp16 partial outputs in f64, add b_proj.

All matmul operands are fp16 (1 cycle/row on the PE at any tile size, vs
~1.9 for f32r and 4x for f32); PSUM accumulation stays fp32.  Attention
blocks run largest-first, with batch-1 QKV tiles interleaved inside the
first (longest) attention block so the PE never idles and the kernel
tail is the smallest block.
"""

import sys

sys.path.insert(0, "/opt/trn_rl_repo")

import numpy as np

B, T, C, H, HD = 2, 2048, 1024, 16, 64
BT = B * T
NCORE = 8
HPC = H // NCORE  # heads per core
NT = BT // 512    # T-tiles for qkv projection
CCH = C // 128    # contraction chunks

_CACHE = {}


def _build():
    if "nc" in _CACHE:
        return _CACHE["nc"]
    from contextlib import ExitStack

    import concourse.bass as bass
    import concourse.bacc as bacc
    import concourse.mybir as mybir
    import concourse.tile as tile
    from concourse.masks import make_identity, make_upper_triangular

    f32, f32r, f16 = mybir.dt.float32, mybir.dt.float32r, mybir.dt.float16
    AF = mybir.ActivationFunctionType

    nc = bacc.Bacc(None, target_bir_lowering=False, debug=False)
    # x pre-permuted on host to the exact SBUF image [p, tt, cc, t] so each
    # T-tile DMA reads contiguous runs per partition
    xT_d = nc.dram_tensor("xT", [128, NT, CCH, 512], f16, kind="ExternalInput")
    wqkv_d = nc.dram_tensor("wqkv", [128, CCH, 3 * 128], f16, kind="ExternalInput")
    bqkv_d = nc.dram_tensor("bqkv", [128, 3], f32, kind="ExternalInput")
    wp_d = nc.dram_tensor("wp", [128, C], f16, kind="ExternalInput")
    out_d = nc.dram_tensor("out", [BT, C], f16, kind="ExternalOutput")

    with tile.TileContext(nc) as tc, ExitStack() as ctx:
        sb = ctx.enter_context(tc.tile_pool(name="sb", bufs=1))
        xp = ctx.enter_context(tc.tile_pool(name="xp", bufs=3))
        vtp = ctx.enter_context(tc.tile_pool(name="vtp", bufs=2))
        esp = ctx.enter_context(tc.tile_pool(name="esp", bufs=6))
        ytp = ctx.enter_context(tc.tile_pool(name="ytp", bufs=3))
        dnp = ctx.enter_context(tc.tile_pool(name="dnp", bufs=4))
        outp = ctx.enter_context(tc.tile_pool(name="outp", bufs=3))
        # PSUM: pa 'mm' 2x[128,512]f32 = 2 banks, pss 2x[128,1024] = 4,
        # pso 2x[65,512] = 2 -> 8 banks total.  All transient tiles (qkv
        # chains, V transposes, proj, denom broadcast) share the 'mm' ring
        # so interleaved qkv tiles never wait on live po accumulators.
        pa = ctx.enter_context(tc.tile_pool(name="pa", bufs=2, space="PSUM"))
        pss = ctx.enter_context(tc.tile_pool(name="pss", bufs=2, space="PSUM"))
        pso = ctx.enter_context(tc.tile_pool(name="pso", bufs=2, space="PSUM"))

        wq_sb = sb.tile([128, CCH, 3 * 128], f16, tag="wq")

        qT = [sb.tile([128, T], f16, tag=f"qT{b}", name=f"qT{b}") for b in range(B)]
        kT = [sb.tile([128, T], f16, tag=f"kT{b}", name=f"kT{b}") for b in range(B)]
        # V' per batch: per k-tile [V_h0 (64) | 1 | V_h1 (64) | 1] = 130 cols;
        # the ones column accumulates the softmax denominator during att@v
        VW = 130
        Vp = [
            sb.tile([128, 16, VW], f16, tag=f"Vp{b}", name=f"Vp{b}") for b in range(B)
        ]
        # ones columns (64, 129) and finite pad from blanket 1.0 fill; V
        # columns are overwritten by the transpose copies.  Vp[1]'s memset is
        # deferred into the first batch-1 qkv tile to keep DVE free at start.
        nc.vector.memset(Vp[0][:, :, :], 1.0)

        # ---------------- QKV projection ----------------
        def emit_qkv_tile(tt):
            b = tt // (NT // B)
            tloc = (tt % (NT // B)) * 512
            if tt == NT // B:
                nc.vector.memset(Vp[1][:, :, :], 1.0)
            x_t = xp.tile([128, CCH, 512], f16, tag="x", name=f"x{tt}")
            if tt == 0:
                # chunked loads on two queue engines so weight and x chunks
                # stream in parallel and the first matmul starts early
                for cc in range(0, CCH, 2):
                    nc.sync.dma_start(
                        out=wq_sb[:, cc : cc + 2, :], in_=wqkv_d[:, cc : cc + 2, :]
                    )
                    nc.gpsimd.dma_start(
                        out=x_t[:, cc : cc + 2, :], in_=xT_d[:, 0, cc : cc + 2, :]
                    )
            else:
                nc.gpsimd.dma_start(out=x_t, in_=xT_d[:, tt, :, :])
            if tt == 0:
                # constants not needed until later: emit their loads after
                # the first x tile so the first matmul starts sooner
                nc.sync.dma_start(out=bias_sb, in_=bqkv_d[:, :])
                nc.sync.dma_start(out=wp_sb, in_=wp_d[:, :])
                make_identity(nc, ident)
                make_upper_triangular(nc, tri2[:, 0, :], val=1.0, diag=True)
                nc.gpsimd.tensor_copy(tri2[:, 1, :], tri2[:, 0, :])
                nc.vector.memset(ones1[:, :], 1.0)
            for g in range(3):
                ps = pa.tile([128, 512], f32, tag="mm", name="ps")
                for cc in range(CCH):
                    nc.tensor.matmul(
                        ps,
                        wq_sb[:, cc, g * 128 : (g + 1) * 128],
                        x_t[:, cc, :],
                        start=(cc == 0),
                        stop=(cc == CCH - 1),
                    )
                if g == 0:
                    nc.vector.tensor_scalar_add(
                        qT[b][:, tloc : tloc + 512], ps, bias_sb[:, 0:1]
                    )
                elif g == 1:
                    nc.vector.tensor_scalar_add(
                        kT[b][:, tloc : tloc + 512], ps, bias_sb[:, 1:2]
                    )
                else:
                    v_t = vtp.tile([128, 512], f16, tag="v")
                    nc.vector.tensor_scalar_add(v_t, ps, bias_sb[:, 2:3])
                    for j in range(4):
                        pt = pa.tile([128, 128], f16, tag="mm", name="pt")
                        nc.tensor.transpose(pt, v_t[:, j * 128 : (j + 1) * 128], ident)
                        ktl = (tt % (NT // B)) * 4 + j
                        # one strided copy moves both heads' V columns
                        nc.vector.tensor_copy(
                            Vp[b][:, ktl, 0:130].rearrange("p (s e) -> p s e", s=2)[
                                :, :, 0:64
                            ],
                            pt[:, :].rearrange("p (s e) -> p s e", s=2),
                        )

        bias_sb = sb.tile([128, 3], f32, tag="bias")
        wp_sb = sb.tile([128, C], f16, tag="wp")
        ident = sb.tile([128, 128], f16, tag="ident")
        tri2 = sb.tile([128, 2, 128], f16, tag="tri2")
        ones1 = sb.tile([1, 64], f16, tag="ones1")

        # ------------- attention + output projection -------------
        scale = 1.0 / 8.0  # 1/sqrt(HD)
        pending_proj = []

        def emit_proj(ytb, b, qb):
            for j in range(4):
                out_t = outp.tile([128, C], f16, tag="out", name="out_t")
                js = slice(j * 128, (j + 1) * 128)
                for ncol in range(2):
                    cs = slice(ncol * 512, (ncol + 1) * 512)
                    pp = pa.tile([128, 512], f32, tag="mm", name="pp")
                    nc.tensor.matmul(
                        pp, ytb[:, js], wp_sb[:, cs], start=True, stop=True
                    )
                    # GpSimd cannot read PSUM; split the PSUM evacuation
                    # between DVE and ACT (ACT has slack beyond the exps)
                    if ncol == 0:
                        nc.vector.tensor_copy(out_t[:, cs], pp)
                    else:
                        nc.scalar.copy(out_t[:, cs], pp)
                row = b * T + qb * 512 + j * 128
                nc.sync.dma_start(out=out_d[row : row + 128, :], in_=out_t)

        def emit_att_block(b, qb, fillers=()):
            fillers = list(fillers)
            n_kt = 4 * (qb + 1)
            po = [
                pso.tile([65, 512], f32, tag="po", name=f"po{b}{qb}{h}")
                for h in range(2)
            ]
            pend = []  # att@v pipelined two k-tiles behind S/exp
            for lkt in range(n_kt):
                r0 = max(0, (lkt - 4 * qb) * 128)
                ks = slice(lkt * 128, (lkt + 1) * 128)
                qs = slice(qb * 512 + r0, (qb + 1) * 512)
                ps2 = pss.tile([128, 1024], f32, tag="s2")
                nc.tensor.matmul(
                    ps2[:, r0:512], kT[b][0:64, ks], qT[b][0:64, qs],
                    start=True, stop=True,
                )
                nc.tensor.matmul(
                    ps2[:, 512 + r0 : 1024], kT[b][64:128, ks], qT[b][64:128, qs],
                    start=True, stop=True,
                )
                es = esp.tile([128, 1024], f16, tag="es")
                if r0:
                    nc.scalar.activation(
                        es[:, :].rearrange("p (h q) -> p h q", h=2)[:, :, r0:512],
                        ps2[:, :].rearrange("p (h q) -> p h q", h=2)[:, :, r0:512],
                        AF.Exp,
                        scale=scale,
                    )
                else:
                    nc.scalar.activation(es, ps2, AF.Exp, scale=scale)
                if lkt >= 4 * qb:  # diagonal tile: causal mask, both heads
                    nc.gpsimd.tensor_mul(
                        es[:, :].rearrange("p (h q) -> p h q", h=2)[
                            :, :, r0 : r0 + 128
                        ],
                        es[:, :].rearrange("p (h q) -> p h q", h=2)[
                            :, :, r0 : r0 + 128
                        ],
                        tri2[:, :, :],
                    )
                if len(pend) >= 2:
                    for mm in pend.pop(0):
                        nc.tensor.matmul(**mm)
                    if fillers and lkt % 3 == 2:
                        emit_qkv_tile(fillers.pop(0))
                pend.append(
                    [
                        dict(
                            out=po[h][:, r0:512],
                            lhsT=Vp[b][:, lkt, h * 65 : (h + 1) * 65],
                            rhs=es[:, h * 512 + r0 : (h + 1) * 512],
                            start=(lkt == 0),
                            stop=(lkt == n_kt - 1),
                        )
                        for h in range(2)
                    ]
                )
            for grp in pend:
                for mm in grp:
                    nc.tensor.matmul(**mm)
            for f in fillers:
                emit_qkv_tile(f)

            # previous block's projection runs on the PE while DVE inverts
            # this block's denominators
            if pending_proj:
                emit_proj(*pending_proj.pop())

            # softmax normalization: invert the denominator rows, broadcast
            # across partitions with a K=1 matmul, scale y on the copy out
            # custom-DVE reciprocal needs SBUF fp32 input (and partition
            # offsets must be 32-aligned, so two separate tiles)
            den0 = dnp.tile([1, 512], f32, tag="den0", name="den0")
            den1 = dnp.tile([1, 512], f32, tag="den1", name="den1")
            nc.vector.tensor_copy(den0, po[0][64:65, :])
            nc.vector.tensor_copy(den1, po[1][64:65, :])
            rcf0 = dnp.tile([1, 512], f32, tag="rcf0", name="rcf0")
            rcf1 = dnp.tile([1, 512], f32, tag="rcf1", name="rcf1")
            nc.vector.reciprocal_approx_fast(out=rcf0, in_=den0)
            nc.vector.reciprocal_approx_fast(out=rcf1, in_=den1)
            rc0 = dnp.tile([1, 512], f16, tag="rc", name="rc0")
            rc1 = dnp.tile([1, 512], f16, tag="rc", name="rc1")
            nc.vector.tensor_copy(rc0, rcf0)
            nc.vector.tensor_copy(rc1, rcf1)
            bc = pa.tile([128, 512], f32, tag="mm", name="bc")
            nc.tensor.matmul(bc[0:64, :], ones1[0:1, :], rc0, start=True, stop=True)
            nc.tensor.matmul(bc[64:128, :], ones1[0:1, :], rc1, start=True, stop=True)
            # DVE cannot read two PSUM operands; stage the broadcast in SBUF
            bcs = dnp.tile([128, 512], f16, tag="bcs", name="bcs")
            nc.vector.tensor_copy(bcs, bc)
            ytb = ytp.tile([128, 512], f16, tag="ytb", name="ytb")
            nc.vector.tensor_mul(ytb[0:64, :], po[0][0:64, :], bcs[0:64, :])
            nc.vector.tensor_mul(ytb[64:128, :], po[1][0:64, :], bcs[64:128, :])
            pending_proj.append((ytb, b, qb))

        # Schedule: batch-0 qkv, then attention blocks largest-first with
        # batch-1 qkv tiles interleaved inside the first (16 k-tile) block.
        for tt in range(4):
            emit_qkv_tile(tt)
        emit_att_block(0, 3, fillers=(4, 5, 6, 7))
        emit_att_block(1, 3)
        emit_att_block(0, 2)
        emit_att_block(1, 2)
        emit_att_block(0, 1)
        emit_att_block(1, 1)
        emit_att_block(0, 0)
        emit_att_block(1, 0)
        emit_proj(*pending_proj.pop())

    nc.finalize()
    _CACHE["nc"] = nc
    return nc


def _prep_inputs(x, w_attn, b_attn, w_proj):
    x = np.ascontiguousarray(np.asarray(x, dtype=np.float32))
    w_attn = np.asarray(w_attn, dtype=np.float32)
    b_attn = np.asarray(b_attn, dtype=np.float32)
    w_proj = np.asarray(w_proj, dtype=np.float32)

    # [p, tt, cc, t] image: xT[p, tt, cc, t] = x_flat[tt*512+t, cc*128+p]
    xT = np.ascontiguousarray(
        x.reshape(NT, 512, CCH, 128).transpose(3, 0, 2, 1).astype(np.float16)
    )
    in_maps = []
    for c in range(NCORE):
        hs = [HPC * c + j for j in range(HPC)]
        blocks = []
        bias_cols = []
        for off in (0, C, 2 * C):
            for h in hs:
                blocks.append(w_attn[:, off + h * HD : off + (h + 1) * HD])
            bias_cols.append(
                np.concatenate([b_attn[off + h * HD : off + (h + 1) * HD] for h in hs])
            )
        wq_flat = np.concatenate(blocks, axis=1).astype(np.float16)  # [C, 384]
        wqkv = np.ascontiguousarray(wq_flat.reshape(CCH, 128, 3 * 128).transpose(1, 0, 2))
        bqkv = np.ascontiguousarray(np.stack(bias_cols, axis=1), dtype=np.float32)
        wp = np.ascontiguousarray(
            np.concatenate([w_proj[h * HD : (h + 1) * HD, :] for h in hs], axis=0)
        ).astype(np.float16)  # [128, C]
        in_maps.append({"xT": xT, "wqkv": wqkv, "bqkv": bqkv, "wp": wp})
    return in_maps


def _run(x, w_attn, b_attn, w_proj, b_proj, trace=False, tmpdir=None):
    from concourse.bass_utils import run_bass_kernel_spmd

    nc = _build()
    in_maps = _prep_inputs(x, w_attn, b_attn, w_proj)
    res = run_bass_kernel_spmd(
        nc, in_maps, list(range(NCORE)), trace=trace, tmpdir=tmpdir
    )
    acc = np.sum(
        np.stack([res.results[i]["out"] for i in range(NCORE)]), axis=0, dtype=np.float64
    )
    out = (acc + np.asarray(b_proj, dtype=np.float64)).astype(np.float32)
    return out.reshape(B, T, C), res


def kernel(x, w_attn, b_attn, w_proj, b_proj):
    out, _ = _run(x, w_attn, b_attn, w_proj, b_proj, trace=False)
    return out
